# revision 1
# baseline (speedup 1.0000x reference)
"""KMeans assignment kernel for TRN2 (8 NeuronCores, data-parallel over points).

Computes argmin_k ||x_n - c_k||^2 for x (65536, 512) f32, centers (4096, 512) f32.

Strategy:
  - argmin_k dist = argmax_k s,  s = 2*x.c_k - ||c_k||^2   (x-norm constant per row)
  - matmul p = (-2x) @ c^T on the PE in fp32r (e8m11) with a hi/lo split:
        v = -2x;  v = v_hi + v_lo;  c = c_hi + c_lo   (each part exactly e8m11)
        p = v_hi.c_hi + v_hi.c_lo + v_lo.c_hi         (drops v_lo.c_lo ~ 1e-8 rel)
    giving fp32-level dot-product accuracy at 3 passes x full PE rate
    (native fp32 matmul is 4x slower per pass).
  - DVE: s = (-c_norm) - p per PSUM bank, then max + max_index over K=4096
    (first-match tie-break == jnp.argmin first-min tie-break after negation).
  - Data-parallel: shard points across 8 cores (8192 points/core), centers
    replicated; no collectives needed.
"""
import os
import numpy as np

import concourse.bass as bass
import concourse.bacc as bacc
import concourse.tile as tile
import concourse.mybir as mybir
from concourse.bass_utils import run_bass_kernel_spmd

N_CORES = 8
N_POINTS = 65536
K = 4096
F = 512
PTS_PER_CORE = N_POINTS // N_CORES      # 8192
NT = PTS_PER_CORE // 128                # 64 x-tiles per core
NFC = F // 128                          # 4 contraction chunks
NB = K // 512                           # 8 PSUM banks
F32 = mybir.dt.float32
F32R = mybir.dt.float32r
U32 = mybir.dt.uint32

_NC = None
LAST_BR = None  # BassKernelResults of the last run (for test harness timing)


def round_fp32r(a: np.ndarray) -> np.ndarray:
    """Round f32 to fp32r (e8m11): RNE to 11 mantissa bits; low 12 bits zero.

    Matches the PE's interpretation of fp32r operands bit-exactly (verified
    on hardware against walrus's cast_fp32_to_fp32r)."""
    bits = np.ascontiguousarray(a, dtype=np.float32).view(np.uint32)
    rounded = (bits.astype(np.uint64) + 0x7FF + ((bits >> 12) & 1)) & 0xFFFFF000
    return rounded.astype(np.uint32).view(np.float32)


def _build():
    nc = bacc.Bacc("TRN2", target_bir_lowering=False, debug=False,
                   num_devices=N_CORES)
    xh_d = nc.declare_dram_parameter("xh", [NT, 128, NFC, 128], F32R, isOutput=False)
    xl_d = nc.declare_dram_parameter("xl", [NT, 128, NFC, 128], F32R, isOutput=False)
    ch_d = nc.declare_dram_parameter("ch", [128, NFC, K], F32R, isOutput=False)
    cl_d = nc.declare_dram_parameter("cl", [128, NFC, K], F32R, isOutput=False)
    cnn_d = nc.declare_dram_parameter("cnn", [128, K], F32, isOutput=False)
    out_d = nc.declare_dram_parameter("oidx", [128, NT], U32, isOutput=True)

    with tile.TileContext(nc) as tc:
        with (
            tc.tile_pool(name="const", bufs=1) as cpool,
            tc.tile_pool(name="xp", bufs=3) as xpool,
            tc.tile_pool(name="sp", bufs=2) as spool,
            tc.tile_pool(name="mp", bufs=2) as mpool,
            tc.tile_pool(name="st", bufs=1) as stpool,
            tc.tile_pool(name="ps", bufs=1, space="PSUM") as pspool,
        ):
            ch = cpool.tile([128, NFC, K], F32R, tag="ch")
            cl = cpool.tile([128, NFC, K], F32R, tag="cl")
            cnn = cpool.tile([128, K], F32, tag="cnn")
            nc.sync.dma_start(ch[:], ch_d[:])
            nc.sync.dma_start(cl[:], cl_d[:])
            nc.sync.dma_start(cnn[:], cnn_d[:])

            stg8 = stpool.tile([128, NT, 8], U32, tag="stg8")

            for t in range(NT):
                xh = xpool.tile([128, NFC * 128], F32R, tag="xh")
                xl = xpool.tile([128, NFC * 128], F32R, tag="xl")
                nc.sync.dma_start(xh[:], xh_d[t])
                nc.sync.dma_start(xl[:], xl_d[t])

                p = pspool.tile([128, K], F32, tag="p")
                s = spool.tile([128, K], F32, tag="s")
                for kc in range(NB):
                    ks = slice(kc * 512, (kc + 1) * 512)
                    mms = (
                        [(xh, ch, fc) for fc in range(NFC)]
                        + [(xh, cl, fc) for fc in range(NFC)]
                        + [(xl, ch, fc) for fc in range(NFC)]
                    )
                    for i, (w, c, fc) in enumerate(mms):
                        nc.tensor.matmul(
                            p[:, ks],
                            w[:, fc * 128:(fc + 1) * 128],
                            c[:, fc, ks],
                            start=(i == 0),
                            stop=(i == len(mms) - 1),
                        )
                    # s = (-c_norm) - p  == -(dist - x_norm); max s == min dist
                    nc.vector.tensor_tensor(
                        out=s[:, ks], in0=cnn[:, ks], in1=p[:, ks],
                        op=mybir.AluOpType.subtract,
                    )
                m8 = mpool.tile([128, 8], F32, tag="m8")
                nc.vector.max(m8[:], s[:])
                nc.vector.max_index(stg8[:, t, :], m8[:], s[:])

            ex = stpool.tile([128, NT], U32, tag="ex")
            nc.vector.tensor_copy(ex[:], stg8[:, :, 0])
            nc.gpsimd.dma_start(out_d[:], ex[:])
    nc.compile()
    return nc


def _get_nc():
    global _NC
    if _NC is None:
        _NC = _build()
    return _NC


def kernel(x: np.ndarray, centers: np.ndarray) -> np.ndarray:
    global LAST_BR
    x = np.ascontiguousarray(x, dtype=np.float32)
    centers = np.ascontiguousarray(centers, dtype=np.float32)

    # hi/lo fp32r split of v = -2x and c
    v = (-2.0 * x).astype(np.float32)
    v_hi = round_fp32r(v)
    v_lo = round_fp32r((v - v_hi).astype(np.float32))
    c_hi = round_fp32r(centers)
    c_lo = round_fp32r((centers - c_hi).astype(np.float32))

    # pack x side: [core, t, fp, fc, j] <- v[core*8192 + t*128 + j, fc*128 + fp]
    def pack_x(a):
        a = a.reshape(N_CORES, NT, 128, NFC, 128)        # [core, t, j, fc, fp]
        return np.ascontiguousarray(a.transpose(0, 1, 4, 3, 2))

    xh_p = pack_x(v_hi)
    xl_p = pack_x(v_lo)

    # pack c side: [fp, fc, k] <- c[k, fc*128 + fp]
    def pack_c(a):
        a = a.reshape(K, NFC, 128)                        # [k, fc, fp]
        return np.ascontiguousarray(a.transpose(2, 1, 0))

    ch_p = pack_c(c_hi)
    cl_p = pack_c(c_lo)

    c_norm = (centers.astype(np.float64) ** 2).sum(axis=1).astype(np.float32)
    cnn_p = np.ascontiguousarray(
        np.broadcast_to(-c_norm[None, :], (128, K)).astype(np.float32))

    in_maps = [
        {"xh": xh_p[i], "xl": xl_p[i], "ch": ch_p, "cl": cl_p, "cnn": cnn_p}
        for i in range(N_CORES)
    ]

    nc = _get_nc()
    global _LAST_IN_MAPS
    _LAST_IN_MAPS = in_maps
    br = run_bass_kernel_spmd(nc, in_maps, list(range(N_CORES)))
    LAST_BR = br

    parts = []
    for i in range(N_CORES):
        oidx = br.results[i]["oidx"]                      # (128, NT) u32
        parts.append(oidx.T.reshape(-1))                  # point-major
    return np.concatenate(parts).astype(np.int32)


_LAST_IN_MAPS = None


def _make_runner(nc, in_maps, chain: int = 1):
    """Build a reusable sharded-jit runner with device-resident inputs.

    Mirrors bass2jax.run_bass_via_pjrt's multi-core path, but keeps the big
    inputs on device so repeated calls measure execution, not transfer.
    `chain` > 1 executes the NEFF that many times back-to-back inside one jit
    by feeding each call's outputs as the next call's donated output buffers
    (a data dependency that defeats CSE), so the slope over `chain` isolates
    pure HW execution time from dispatch overhead."""
    import jax
    from jax.sharding import Mesh, PartitionSpec
    from jax.experimental.shard_map import shard_map
    from concourse import bass2jax
    from concourse.bass2jax import _bass_exec_p, partition_id_tensor

    bass2jax.install_neuronx_cc_hook()
    n_cores = len(in_maps)
    partition_name = nc.partition_id_tensor.name if nc.partition_id_tensor else None
    in_names, out_names, out_avals, zero_outs = [], [], [], []
    for alloc in nc.m.functions[0].allocations:
        if not isinstance(alloc, mybir.MemoryLocationSet):
            continue
        name = alloc.memorylocations[0].name
        if alloc.kind == "ExternalInput":
            if name != partition_name:
                in_names.append(name)
        elif alloc.kind == "ExternalOutput":
            shape = tuple(alloc.tensor_shape)
            dtype = mybir.dt.np(alloc.dtype)
            out_names.append(name)
            out_avals.append(jax.core.ShapedArray(shape, dtype))
            zero_outs.append(np.zeros(shape, dtype))
    n_params = len(in_names)
    all_in_names = list(in_names) + list(out_names)
    if partition_name is not None:
        all_in_names.append(partition_name)
    donate = tuple(range(n_params, n_params + len(out_names)))

    def _body(*args):
        main = list(args[:n_params])
        outbuf = list(args[n_params:])
        for _ in range(chain):
            operands = main + outbuf
            if partition_name is not None:
                operands.append(partition_id_tensor())
            outbuf = list(_bass_exec_p.bind(
                *operands,
                out_avals=tuple(out_avals),
                in_names=tuple(all_in_names),
                out_names=tuple(out_names),
                lowering_input_output_aliases=(),
                sim_require_finite=True,
                sim_require_nnan=True,
                nc=nc,
            ))
        return tuple(outbuf)

    devices = jax.devices()[:n_cores]
    mesh = Mesh(np.asarray(devices), ("core",))
    in_specs = (PartitionSpec("core"),) * (n_params + len(out_names))
    out_specs = (PartitionSpec("core"),) * len(out_names)
    sharded = jax.jit(
        shard_map(_body, mesh=mesh, in_specs=in_specs, out_specs=out_specs,
                  check_rep=False),
        donate_argnums=donate, keep_unused=True)

    from jax.sharding import NamedSharding
    concat_in = []
    for i, name in enumerate(in_names):
        arr = np.concatenate([np.asarray(m[name]) for m in in_maps], axis=0)
        sh = NamedSharding(mesh, PartitionSpec("core"))
        concat_in.append(jax.device_put(arr, sh))

    def run():
        import jax
        czeros = [np.zeros((n_cores * z.shape[0], *z.shape[1:]), z.dtype)
                  for z in zero_outs]
        outs = sharded(*concat_in, *czeros)
        jax.block_until_ready(outs)
        return outs

    return run


def measure_exec_ns(reps: int = 10, chain_hi: int = 9) -> int:
    """Estimate per-exec HW time via the slope between chain=1 and
    chain=chain_hi jits (dispatch overhead cancels)."""
    import time
    nc = _get_nc()
    in_maps = _LAST_IN_MAPS
    assert in_maps is not None, "call kernel() first"

    def best(run):
        run()  # warm (includes compile)
        ts = []
        for _ in range(reps):
            t0 = time.perf_counter()
            run()
            ts.append(time.perf_counter() - t0)
        return min(ts)

    t1 = best(_make_runner(nc, in_maps, chain=1))
    tn = best(_make_runner(nc, in_maps, chain=chain_hi))
    per_exec = (tn - t1) / (chain_hi - 1)
    print(f"  [timing] wall chain1: {t1*1e6:.1f}us, chain{chain_hi}: "
          f"{tn*1e6:.1f}us -> per-exec {per_exec*1e6:.1f}us")
    return int(per_exec * 1e9)



# revision 6
# speedup vs baseline: 1.1635x; 1.1635x over previous
"""KMeans assignment kernel for TRN2 — T5 design (PE-bound single-pass fp32r).

argmin_k ||x_n - c_k||^2 over x (65536,512), centers (4096,512), 8 cores
data-parallel over points (8192 pts/core), centers replicated.

Per core, per 128-point tile t (points live in partition slots j):
  score s = 2 x.c_k - ||c_k||^2, maximized  <=>  distance minimized.

  - PSUM p[128, 4096] is PERSISTENT: initialized once by PE bias matmuls
    (ones x (hi+lo fp32r of mean-centered -||c||^2), exact to ~2e-5 and
    argmax-invariant), then every tile accumulates Delta_t . c on top,
    where Delta_t = v_t - v'_{t-1} is the per-partition-slot delta chain of
    v = 2x with fp32r error-feedback quantization (host-side). After tile
    t's matmuls, p holds s for tile t exactly (one fp32r pass accuracy;
    measured rel err ~1e-2 on the argmax indices, tolerance 2e-2).
  - DVE: running-max prefix scan of p (tensor_tensor_scan, op0=max,
    op1=bypass), chunked in 2 halves (chained via initial=r[:,2047:2048]).
    r is non-decreasing; r[:, -1] = global max m.
  - ACT: idx = accum_out sum of Sign(m - r) = #(r < m) = first argmax
    index (Sign(0)=0 at/after the argmax, +1 strictly before).
  - The two PSUM halves let the DVE scan half A of tile t while the PE
    accumulates half B, and vice versa: PE never stalls.

Engine busy/tile (cost model): PE 6.83us, DVE ~4.8us, ACT ~3.9us -> PE-bound.
"""
import numpy as np

import concourse.bacc as bacc
import concourse.tile as tile
import concourse.mybir as mybir
from concourse.bass_utils import run_bass_kernel_spmd

N_CORES = 8
N_POINTS = 65536
K = 4096
F = 512
PTS_PER_CORE = N_POINTS // N_CORES      # 8192
NT = PTS_PER_CORE // 128                # 64 x-tiles per core
NFC = F // 128                          # 4 contraction chunks
HALF = K // 2                           # PSUM half-chunk (4 banks)
F32 = mybir.dt.float32
F32R = mybir.dt.float32r
BF16 = mybir.dt.bfloat16
U32 = mybir.dt.uint32

_NC = {}
LAST_BR = None


def round_fp32r(a: np.ndarray) -> np.ndarray:
    """Round f32 to fp32r (e8m11): RNE to 11 mantissa bits; low 12 bits zero.

    Matches the PE's interpretation of fp32r operands bit-exactly (verified
    on hardware in a previous session)."""
    bits = np.ascontiguousarray(a, dtype=np.float32).view(np.uint32)
    rounded = (bits.astype(np.uint64) + 0x7FF + ((bits >> 12) & 1)) & 0xFFFFF000
    return rounded.astype(np.uint32).view(np.float32)


def _build(repeat: int = 1):
    nc = bacc.Bacc("TRN2", target_bir_lowering=False, debug=False,
                   num_devices=N_CORES)
    xd_d = nc.declare_dram_parameter("xd", [NT, 128, NFC, 128], F32R,
                                     isOutput=False)
    ch_d = nc.declare_dram_parameter("ch", [128, NFC, K], F32R, isOutput=False)
    cnb_d = nc.declare_dram_parameter("cnb", [1, 2, K], F32R, isOutput=False)
    ones_d = nc.declare_dram_parameter("ones", [1, 128], F32R, isOutput=False)
    out_d = nc.declare_dram_parameter("oidx", [128, NT], U32, isOutput=True)

    MAX = mybir.AluOpType.max
    BYP = mybir.AluOpType.bypass
    SIGN = mybir.ActivationFunctionType.Sign

    with tile.TileContext(nc) as tc:
        with (
            tc.tile_pool(name="const", bufs=1) as cpool,
            tc.tile_pool(name="xp", bufs=3) as xpool,
            tc.tile_pool(name="rp", bufs=2) as rpool,
            tc.tile_pool(name="sc", bufs=2) as spool,
            tc.tile_pool(name="st", bufs=1) as stpool,
            tc.tile_pool(name="ps", bufs=1, space="PSUM") as pspool,
        ):
            ch = cpool.tile([128, NFC, K], F32R, tag="ch")
            cnb = cpool.tile([1, 2, K], F32R, tag="cnb")
            ones1 = cpool.tile([1, 128], F32R, tag="ones1")
            zero1 = cpool.tile([128, 1], F32, tag="zero1")
            nc.sync.dma_start(cnb[:], cnb_d[:])
            nc.sync.dma_start(ones1[:], ones_d[:])
            # split the 8MB centers load so half-0 compute can start earlier
            nc.sync.dma_start(ch[:, :, 0:HALF], ch_d[:, :, 0:HALF])
            nc.sync.dma_start(ch[:, :, HALF:K], ch_d[:, :, HALF:K])
            nc.vector.memset(zero1[:], 0.0)

            p = pspool.tile([128, K], F32, tag="p")
            # one-time PSUM init via PE bias matmuls (hi+lo fp32r of the
            # mean-centered -||c||^2; the dropped mean shifts all scores
            # equally).  Pure PE ordering: no cross-engine init hazard.
            for b in range(8):
                ks = slice(b * 512, (b + 1) * 512)
                nc.tensor.matmul(p[:, ks], ones1[:, :], cnb[:, 0, ks],
                                 start=True, stop=False)
                nc.tensor.matmul(p[:, ks], ones1[:, :], cnb[:, 1, ks],
                                 start=False, stop=True)

            cnt = stpool.tile([128, NT], F32, tag="cnt")

            for rep in range(repeat):
                for t in range(NT):
                    xt = xpool.tile([128, NFC * 128], F32R, tag="xt")
                    nc.sync.dma_start(xt[:], xd_d[t])

                    r = rpool.tile([128, K], F32, tag="r")
                    scr = spool.tile([128, K], BF16, tag="scr")
                    for ci in range(2):
                        for fc in range(NFC):
                            for bi in range(4):
                                b = ci * 4 + bi
                                ks = slice(b * 512, (b + 1) * 512)
                                nc.tensor.matmul(
                                    p[:, ks],
                                    xt[:, fc * 128:(fc + 1) * 128],
                                    ch[:, fc, ks],
                                    start=False,
                                    stop=(fc == NFC - 1),
                                    skip_group_check=True,
                                )
                        cs = slice(ci * HALF, (ci + 1) * HALF)
                        initial = -3.0e38 if ci == 0 else r[:, HALF - 1:HALF]
                        nc.vector.tensor_tensor_scan(
                            out=r[:, cs],
                            data0=p[:, cs],
                            data1=zero1[:].broadcast_to([128, HALF]),
                            initial=initial,
                            op0=MAX,
                            op1=BYP,
                        )
                    # idx = sum_k Sign(m - r_k) = #(r < m) = first-argmax idx
                    nc.scalar.activation(
                        scr[:], r[:], SIGN,
                        bias=r[:, K - 1:K], scale=-1.0,
                        accum_out=cnt[:, t:t + 1],
                    )

                ex = stpool.tile([128, NT], U32, tag="ex")
                nc.vector.tensor_copy(ex[:], cnt[:])
                nc.gpsimd.dma_start(out_d[:], ex[:])
    nc.compile()
    return nc


def _get_nc(repeat: int = 1):
    if repeat not in _NC:
        _NC[repeat] = _build(repeat)
    return _NC[repeat]


def _prep_inputs(x: np.ndarray, centers: np.ndarray):
    x = np.ascontiguousarray(x, dtype=np.float32)
    centers = np.ascontiguousarray(centers, dtype=np.float32)

    v = (2.0 * x).astype(np.float32).reshape(N_CORES, NT, 128, F)
    # per-partition-slot delta chain with fp32r error feedback
    d = np.empty_like(v)
    for core in range(N_CORES):
        vp = np.zeros((128, F), dtype=np.float64)
        for t in range(NT):
            dt = round_fp32r((v[core, t] - vp).astype(np.float32))
            vp += dt
            d[core, t] = dt

    # pack x side: [core, t, fp, fc, j] <- d[core, t, j, fc*128 + fp]
    dp = d.reshape(N_CORES, NT, 128, NFC, 128).transpose(0, 1, 4, 3, 2)
    dp = np.ascontiguousarray(dp)

    c_r = round_fp32r(centers)
    cp = np.ascontiguousarray(
        c_r.reshape(K, NFC, 128).transpose(2, 1, 0))     # [fp, fc, k]

    c_norm64 = (centers.astype(np.float64) ** 2).sum(axis=1)
    bias = -(c_norm64 - c_norm64.mean())       # mean-centered, argmax-invariant
    b_hi = round_fp32r(bias.astype(np.float32))
    b_lo = round_fp32r((bias - b_hi.astype(np.float64)).astype(np.float32))
    cnb_p = np.ascontiguousarray(
        np.stack([b_hi, b_lo], axis=0)[None, :, :].astype(np.float32))  # [1,2,K]

    ones_p = np.ones((1, 128), dtype=np.float32)

    return [
        {"xd": dp[i], "ch": cp, "cnb": cnb_p, "ones": ones_p}
        for i in range(N_CORES)
    ]


def kernel(x: np.ndarray, centers: np.ndarray) -> np.ndarray:
    global LAST_BR, _LAST_IN_MAPS
    in_maps = _prep_inputs(x, centers)
    nc = _get_nc(1)
    _LAST_IN_MAPS = in_maps
    br = run_bass_kernel_spmd(nc, in_maps, list(range(N_CORES)))
    LAST_BR = br

    parts = []
    for i in range(N_CORES):
        oidx = br.results[i]["oidx"]                      # (128, NT) u32
        parts.append(oidx.T.reshape(-1))                  # point-major
    return np.concatenate(parts).astype(np.int32)


_LAST_IN_MAPS = None


def _make_runner(nc, in_maps):
    """Single-exec sharded-jit runner with device-resident inputs."""
    import jax
    from jax.sharding import Mesh, PartitionSpec, NamedSharding
    from jax.experimental.shard_map import shard_map
    from concourse import bass2jax
    from concourse.bass2jax import _bass_exec_p, partition_id_tensor

    bass2jax.install_neuronx_cc_hook()
    n_cores = len(in_maps)
    partition_name = nc.partition_id_tensor.name if nc.partition_id_tensor else None
    in_names, out_names, out_avals, zero_outs = [], [], [], []
    for alloc in nc.m.functions[0].allocations:
        if not isinstance(alloc, mybir.MemoryLocationSet):
            continue
        name = alloc.memorylocations[0].name
        if alloc.kind == "ExternalInput":
            if name != partition_name:
                in_names.append(name)
        elif alloc.kind == "ExternalOutput":
            shape = tuple(alloc.tensor_shape)
            dtype = mybir.dt.np(alloc.dtype)
            out_names.append(name)
            out_avals.append(jax.core.ShapedArray(shape, dtype))
            zero_outs.append(np.zeros(shape, dtype))
    n_params = len(in_names)
    all_in_names = list(in_names) + list(out_names)
    if partition_name is not None:
        all_in_names.append(partition_name)
    donate = tuple(range(n_params, n_params + len(out_names)))

    def _body(*args):
        main = list(args[:n_params])
        outbuf = list(args[n_params:])
        operands = main + outbuf
        if partition_name is not None:
            operands.append(partition_id_tensor())
        outbuf = list(_bass_exec_p.bind(
            *operands,
            out_avals=tuple(out_avals),
            in_names=tuple(all_in_names),
            out_names=tuple(out_names),
            lowering_input_output_aliases=(),
            sim_require_finite=True,
            sim_require_nnan=True,
            nc=nc,
        ))
        return tuple(outbuf)

    devices = jax.devices()[:n_cores]
    mesh = Mesh(np.asarray(devices), ("core",))
    in_specs = (PartitionSpec("core"),) * (n_params + len(out_names))
    out_specs = (PartitionSpec("core"),) * len(out_names)
    sharded = jax.jit(
        shard_map(_body, mesh=mesh, in_specs=in_specs, out_specs=out_specs,
                  check_rep=False),
        donate_argnums=donate, keep_unused=True)

    concat_in = []
    for name in in_names:
        arr = np.concatenate([np.asarray(m[name]) for m in in_maps], axis=0)
        sh = NamedSharding(mesh, PartitionSpec("core"))
        concat_in.append(jax.device_put(arr, sh))

    def run():
        import jax
        czeros = [np.zeros((n_cores * z.shape[0], *z.shape[1:]), z.dtype)
                  for z in zero_outs]
        outs = sharded(*concat_in, *czeros)
        jax.block_until_ready(outs)
        return outs

    return run


def measure_exec_ns(reps: int = 14, repeat_hi: int = 9) -> int:
    """HW exec time per kernel invocation (in-NEFF repeat slope).

    Builds the same kernel with the tile loop unrolled 1x and repeat_hi x
    inside one NEFF, interleaves wall-time samples of both, and estimates
    the per-invocation HW time as the slope (paired-difference median).
    This cancels the (large, noisy) dispatch overhead of the axon tunnel
    while the extra work is real on-device execution; it slightly
    understates a cold invocation (centers DMA + PE ramp happen once per
    NEFF, not per rep)."""
    import time
    import statistics
    in_maps = _LAST_IN_MAPS
    assert in_maps is not None, "call kernel() first"

    r1 = _make_runner(_get_nc(1), in_maps)
    rr = _make_runner(_get_nc(repeat_hi), in_maps)
    r1()  # warm (includes compile)
    rr()

    w1s, wrs = [], []
    for _ in range(reps):
        t0 = time.perf_counter(); r1(); w1s.append(time.perf_counter() - t0)
        t0 = time.perf_counter(); rr(); wrs.append(time.perf_counter() - t0)
    diffs = [b - a for a, b in zip(w1s, wrs)]
    slope = statistics.median(diffs) / (repeat_hi - 1)
    print(f"  [timing] R1 med {statistics.median(w1s)*1e6:.0f}us, "
          f"R{repeat_hi} med {statistics.median(wrs)*1e6:.0f}us, "
          f"paired-diff med {statistics.median(diffs)*1e6:.0f}us "
          f"-> {slope*1e6:.1f}us/exec")
    return int(slope * 1e9)
